# revision 4
# baseline (speedup 1.0000x reference)
"""CRF loss kernel for Trainium2 (8 NeuronCores, Bass/Tile).

Math
----
The reference computes, for one sequence of SEQ=16384 steps over
TAG=1024 tags:

  forward:  fv_{t+1}[j] = logsumexp_i(fv_t[i] + T[j,i]) + feat_t[j]
  score    = logsumexp_j(fv_SEQ[j] + T[stop,j]);  out = score - gold

In real space with E = exp(T) the recurrence is p_{t+1} = exp(feat_t) *
(E @ p_t).  Products of positive random matrices forget their initial
direction at ~e^-2.5/step (top-two-singular-value ratio ~12), so the
16384-step chain splits into 1024 independent chunks of L=16 steps,
every chunk started from the all-ones vector with NO warm-up: the
chunk-start 1-norm is then exactly TAG, and the per-chunk growth ratios
telescope to the true log-norm (measured end-to-end rel err 1e-3 at
fp8-e5m2, 1e-6 at bf16, vs. the 2e-2 gate).  Chunk 0 (which needs the
exact one-hot start) runs on the host in f64 — 16 matvecs.

Device program (per core, 128 chains, 16 lockstep steps)
-------------------------------------------------------
All operands are fp8-e5m2; matmuls use DoubleRow perf mode (two 128-tag
blocks contracted per pass).  State X[j, b] keeps tags on partitions and
chains on the free dim, split into TWO chain-streams of 64 whose fused
X' = q * fe DVE muls hide under the other stream's PE matmuls:

  q[j',b]  = sum_j Mhat[j,j'] X[j,b]   8 groups x 4 DoubleRow matmuls
        stationary = Mhat block [128, 2, 128]  (resident in SBUF)
        moving     = X pair view [128, 2, 64]
  X'[j',b] = q * FE_s[j',b]            ONE [128,512] DVE mul per stream

Per-step chain-norm records are ones-column matmuls; all heavy inputs
(Mhat = exp(T^T - DELTA), FE = pre-exp'd per-step feat tiles in device
layout) are prepared on the host, DMA'd once, and stay resident — the
steady loop issues no DMA, no transposes, no PSUM->SBUF state copies.
Dummy matmuls pre-warm the PE p-state ramp during the boot DMA.  The
gold score is O(seq + tag) index gathers, computed on the host.
"""

import os
import sys
import numpy as np
import ml_dtypes

for _p in ("/opt/trn_rl_repo",):
    if _p not in sys.path:
        sys.path.insert(0, _p)

from contextlib import ExitStack

from concourse import bacc, tile
from concourse import mybir
from concourse.bass_utils import run_bass_kernel_spmd

F32 = mybir.dt.float32
BF16 = mybir.dt.bfloat16
BF16_NP = ml_dtypes.bfloat16
FP8 = mybir.dt.float8e5
FP8_NP = ml_dtypes.float8_e5m2
NPAIR = 4          # K-pairs per step (DoubleRow: 2 tag-blocks per matmul)

SEQ = 16384
TAG = 1024
P = 128            # partitions / PE tile edge / chains per core
NT = TAG // P      # 8 tag tiles
NCORES = 8
L = 16             # chunk length (steps per chunk)
K = 0              # warm-up steps per chain (chunk-start norm is exactly TAG)
LEN = L + K        # lockstep steps per core
DELTA = 8.0        # per-step log-growth folded into Mhat
CHAINS = SEQ // L  # 1024 global chains

# ft DMA chunks: step ranges whose FE tiles arrive in one DMA each
FT_CHUNKS = [(0, 1), (1, 3), (3, 9), (9, LEN)]  # LEN == 16

_compiled = None
LAST_RESULTS = None


def _build_kernel():
    nc = bacc.Bacc(
        "TRN2",
        target_bir_lowering=False,
        debug=False,
        num_devices=NCORES,
    )

    # mh layout is jt-major: block jt holds Mhat[:, jt*128:(jt+1)*128] as
    # [128 (k partition), 8 kt x 128 (j')] so each group's weights arrive in
    # one DMA.  Block 0 rides in `boot` (one DMA covers everything the first
    # matmul group needs: ucol | ones | mh block 0); the all-ones init state
    # is memset on device (chunk 0's exact 16-step prefix runs on the host).
    BOOT_W = NT + 1 + TAG
    boot = nc.declare_dram_parameter("boot", [P, BOOT_W], FP8, isOutput=False)
    mh = nc.declare_dram_parameter("mh", [P, NT * TAG], FP8, isOutput=False)
    ft = nc.declare_dram_parameter("ft", [P, LEN * TAG], FP8, isOutput=False)

    sums = nc.declare_dram_parameter("sums", [1, 4 * P], F32, isOutput=True)

    with tile.TileContext(nc) as tc, ExitStack() as ctx:
        cpool = ctx.enter_context(tc.tile_pool(name="cpool", bufs=1))
        xpool = ctx.enter_context(tc.tile_pool(name="xpool", bufs=2))
        qpool = ctx.enter_context(
            tc.tile_pool(name="qpool", bufs=1, space="PSUM"))
        rpool = ctx.enter_context(
            tc.tile_pool(name="rpool", bufs=1, space="PSUM"))

        recs_sb = cpool.tile([1, 4 * P], F32)
        boot_t = cpool.tile([P, BOOT_W], FP8)

        # ---- staged input DMAs on two HWDGE queues (SP + Act)
        nc.sync.dma_start(boot_t[:], boot[:])
        ucol_t = boot_t[:, 0:NT]
        ones_t = boot_t[:, NT:NT + 1]
        # warm-up operand first so the PE dummies start immediately; the
        # (bigger) state-init memset rides on the otherwise idle Pool engine.
        warm_sb = cpool.tile([P, P], BF16)
        nc.vector.memset(warm_sb[:], 0.0)
        xinit = cpool.tile([P, TAG], FP8)
        nc.gpsimd.memset(xinit[:], 1.0)
        xt = [xinit[:, 2 * p * P:2 * (p + 1) * P] for p in range(NPAIR)]

        ft_t = []                     # one tile per chunk
        ft_of = {}                    # step -> (tile, col offset)
        for ci, (s0, s1) in enumerate(FT_CHUNKS):
            tchunk = cpool.tile([P, (s1 - s0) * TAG], FP8, tag=f"ft{ci}",
                                name=f"ft{ci}")
            ft_t.append(tchunk)
            for s in range(s0, s1):
                ft_of[s] = (tchunk, (s - s0) * TAG)

        mh_t = [boot_t[:, NT + 1:NT + 1 + TAG]]
        mh_rest = [cpool.tile([P, TAG], FP8, tag=f"mh{jt}", name=f"mh{jt}")
                   for jt in range(1, NT)]
        mh_t.extend(mh_rest)
        # arrival order: all mh blocks first (step-0 matmuls need only mh +
        # memset state; the first mul needs ft chunk 0 only after a full PE
        # group) — alternated across the SP and Act queues so real hardware
        # loads them in parallel.
        for jt in range(1, NT):
            eng = nc.sync if jt % 2 == 1 else nc.scalar
            eng.dma_start(mh_t[jt][:], mh[:, jt * TAG:(jt + 1) * TAG])
        for ci in range(len(FT_CHUNKS)):
            s0, s1 = FT_CHUNKS[ci]
            nc.scalar.dma_start(ft_t[ci][:], ft[:, s0 * TAG:s1 * TAG])

        # ---- PE pre-warm: dummy matmuls with no DMA deps keep the PE busy
        # through the boot DMA so the pstate ramp completes before step 0.
        warm_ps = rpool.tile([P, P], F32, tag="warm")
        for _ in range(28):
            nc.tensor.matmul(warm_ps[:], lhsT=warm_sb[:], rhs=warm_sb[:],
                             start=True, stop=True)

        rec_slot = {LEN - 1: 2}

        DR = mybir.MatmulPerfMode.DoubleRow
        HB = P // 2    # chains per stream

        def pairs_of(ap2d):
            return ap2d.rearrange("a (two f) -> a two f", two=2)

        # Two interleaved chain-streams (b 0..63 / 64..127): each stream's
        # X'=q*fe mul (ONE fused [128,512] DVE op) hides under the other
        # stream's PE matmuls.  Per-stream state tile layout: [blk(8) x 64].
        # step-0 state is all-ones: any slice of xinit serves as a pair.
        xt_s = [xinit[:, 0:NT * HB], xinit[:, NT * HB:2 * NT * HB]]

        def pair_view(xs, p):
            return pairs_of(xs[:, 2 * p * HB:2 * (p + 1) * HB])

        for s in range(LEN):
            fch, fo = ft_of[s]
            nxt = [None, None]
            for strm in range(2):
                q = qpool.tile([P, NT * HB], F32, tag=f"q{strm}",
                               name=f"q{strm}", bufs=2)
                for jt in range(NT):
                    for p in range(NPAIR):
                        nc.tensor.matmul(
                            q[:, jt * HB:(jt + 1) * HB],
                            lhsT=pairs_of(
                                mh_t[jt][:, 2 * p * P:2 * (p + 1) * P]),
                            rhs=pair_view(xt_s[strm], p),
                            start=(p == 0), stop=(p == NPAIR - 1),
                            perf_mode=DR)
                xq = xpool.tile([P, NT * HB], FP8, tag=f"xq{strm}",
                                name=f"xq{strm}")
                # host ft layout [s][strm][blk(8)][64] -> one contiguous slice
                ftv = fch[:, fo + strm * NT * HB:fo + (strm + 1) * NT * HB]
                nc.vector.tensor_mul(xq[:], q[:], ftv)
                nxt[strm] = xq
            xt_s = nxt
            if s in rec_slot:
                rec = rpool.tile([1, P], F32, tag="rec")
                for strm in range(2):
                    for kt in range(NT):
                        nc.tensor.matmul(
                            rec[:, strm * HB:(strm + 1) * HB],
                            lhsT=ones_t[:],
                            rhs=nxt[strm][:, kt * HB:(kt + 1) * HB],
                            start=(kt == 0), stop=(kt == NT - 1))
                slot = rec_slot[s]
                nc.scalar.copy(recs_sb[:, slot * P:(slot + 1) * P], rec[:])

        dots = rpool.tile([1, P], F32, tag="rec")
        for strm in range(2):
            for kt in range(NT):
                nc.tensor.matmul(
                    dots[:, strm * HB:(strm + 1) * HB],
                    lhsT=ucol_t[:, kt:kt + 1],
                    rhs=xt_s[strm][:, kt * HB:(kt + 1) * HB],
                    start=(kt == 0), stop=(kt == NT - 1))
        nc.scalar.copy(recs_sb[:, 3 * P:4 * P], dots[:])
        nc.sync.dma_start(sums[:], recs_sb[:])

    nc.compile()
    return nc


def _prep_inputs(feats, T, start_i):
    """Host-side: Mhat (jt-major blocks), pre-exp'd per-step feat tiles,
    init state."""
    mhat = np.exp(T.T.astype(np.float64) - DELTA).astype(np.float32)  # [j, j']
    # block jt: [128 (k part), NT kt x 128] with element [i, kt*128 + c] =
    # Mhat[kt*128 + i, jt*128 + c]
    mh_sb = np.ascontiguousarray(
        mhat.reshape(NT, P, NT, P)      # [kt, i, jt, c]
        .transpose(1, 2, 0, 3)          # [i, jt, kt, c]
        .reshape(P, NT * TAG)).astype(FP8_NP)

    fe = np.exp(feats.astype(np.float32)).astype(FP8_NP)  # [SEQ, TAG]

    in_maps = []
    for g in range(NCORES):
        # chain c = 128g + b covers rows [16c, 16c+16); warmup row 16c-K.
        b = np.arange(P)
        rows = (L * (P * g + b))[None, :] + np.arange(LEN)[:, None]
        ftg = fe[rows]                                  # [LEN, 128(b), 1024(j)]
        ftg = ftg.transpose(0, 2, 1).reshape(LEN, NT, P, P)   # [s, jt, j_l, b]
        # device layout: [j_l part, s, strm(2), blk(8), b_local(64)]
        HB = P // 2
        ftg = ftg.reshape(LEN, NT, P, 2, HB)      # [s, blk, j_l, strm, bl]
        ft_sb = np.ascontiguousarray(
            ftg.transpose(2, 0, 3, 1, 4).reshape(P, LEN * TAG))

        in_maps.append({"mh": mh_sb, "ft": ft_sb})
    return in_maps


def kernel(feats, transitions, tags, start_idx, stop_idx):
    global _compiled, LAST_RESULTS
    feats = np.ascontiguousarray(np.asarray(feats, dtype=np.float32))
    T = np.ascontiguousarray(np.asarray(transitions, dtype=np.float32))
    tags_np = np.asarray(tags).astype(np.int64)
    start_i = int(np.asarray(start_idx))
    stop_i = int(np.asarray(stop_idx))

    in_maps = _prep_inputs(feats, T, start_i)
    u = np.exp(T[stop_i].astype(np.float64)).astype(np.float32)
    ucol_sb = np.ascontiguousarray(u.reshape(NT, P).T).astype(FP8_NP)
    ones_sb = np.ones((P, 1), FP8_NP)
    for m in in_maps:
        # boot = ucol | ones | mh block 0
        m["boot"] = np.ascontiguousarray(np.concatenate(
            [ucol_sb, ones_sb, m["mh"][:, 0:TAG]], axis=1))

    # chunk 0's exact 16-step prefix in f64 on the host (16 matvecs):
    # anchors the absolute scale that all other chunks telescope from.
    E64 = np.exp(T.astype(np.float64))
    w = np.zeros(TAG, np.float64)
    w[start_i] = 1.0
    fe64 = np.exp(feats[:L].astype(np.float64))
    for t in range(L):
        w = fe64[t] * (E64 @ w)
    logw16 = float(np.log(w.sum()))

    if _compiled is None:
        _compiled = _build_kernel()
    res = run_bass_kernel_spmd(
        _compiled, in_maps, list(range(NCORES)),
        trace=bool(os.environ.get("KERNEL_TRACE")))
    LAST_RESULTS = res
    results = res.results

    # ---- host stitch (~3k scalars)
    sums_by_core = [results[g]["sums"].reshape(4, P) for g in range(NCORES)]
    end = np.concatenate(
        [sums_by_core[g][2] for g in range(NCORES)]).astype(np.float64)
    d = float(sums_by_core[NCORES - 1][3][P - 1])

    # chunk-start norm is exactly |ones| = TAG (zero warm-up steps)
    fs = (np.log(d) - np.log(end[CHAINS - 1])
          + float(np.sum(np.log(end[1:]))) - (CHAINS - 1) * np.log(float(TAG))
          + logw16 + (SEQ - L) * DELTA)

    # ---- gold score on host (index gathers, O(seq + tag))
    tags_ext = np.concatenate([np.array([start_i], dtype=np.int64), tags_np])
    gold = (float(T[tags_ext[1:], tags_ext[:-1]].astype(np.float64).sum())
            + feats[tags_ext[1:]].astype(np.float64).sum(axis=0)
            + float(T[stop_i, tags_ext[-1]]))

    return (fs - gold).astype(np.float32)


# revision 5
# speedup vs baseline: 1.0527x; 1.0527x over previous
"""CRF loss kernel for Trainium2 (8 NeuronCores, Bass/Tile).

Math
----
The reference computes, for one sequence of SEQ=16384 steps over
TAG=1024 tags:

  forward:  fv_{t+1}[j] = logsumexp_i(fv_t[i] + T[j,i]) + feat_t[j]
  score    = logsumexp_j(fv_SEQ[j] + T[stop,j]);  out = score - gold

In real space with E = exp(T) the recurrence is p_{t+1} = exp(feat_t) *
(E @ p_t).  Products of positive random matrices forget their initial
direction at ~e^-2.5/step (top-two-singular-value ratio ~12), so the
16384-step chain splits into 1024 independent chunks of L=16 steps,
every chunk started from the all-ones vector with NO warm-up: the
chunk-start 1-norm is then exactly TAG, and the per-chunk growth ratios
telescope to the true log-norm (measured end-to-end rel err 1e-3 at
fp8-e5m2, 1e-6 at bf16, vs. the 2e-2 gate).  Chunk 0 (which needs the
exact one-hot start) runs on the host in f64 — 16 matvecs.

Device program (per core, 128 chains, 16 lockstep steps)
-------------------------------------------------------
All operands are fp8-e5m2; matmuls use DoubleRow perf mode (two 128-tag
blocks contracted per pass).  State X[j, b] keeps tags on partitions and
chains on the free dim, split into TWO chain-streams of 64 whose fused
X' = q * fe DVE muls hide under the other stream's PE matmuls:

  q[j',b]  = sum_j Mhat[j,j'] X[j,b]   8 groups x 4 DoubleRow matmuls
        stationary = Mhat block [128, 2, 128]  (resident in SBUF)
        moving     = X pair view [128, 2, 64]
  X'[j',b] = q * FE_s[j',b]            ONE [128,512] DVE mul per stream

Per-step chain-norm records are ones-column matmuls; all heavy inputs
(Mhat = exp(T^T - DELTA), FE = pre-exp'd per-step feat tiles in device
layout) are prepared on the host, DMA'd once, and stay resident — the
steady loop issues no DMA, no transposes, no PSUM->SBUF state copies.
Dummy matmuls pre-warm the PE p-state ramp during the boot DMA.  The
gold score is O(seq + tag) index gathers, computed on the host.
"""

import os
import sys
import numpy as np
import ml_dtypes

for _p in ("/opt/trn_rl_repo",):
    if _p not in sys.path:
        sys.path.insert(0, _p)

from contextlib import ExitStack

from concourse import bacc, tile
from concourse import mybir
from concourse.bass_utils import run_bass_kernel_spmd

F32 = mybir.dt.float32
BF16 = mybir.dt.bfloat16
BF16_NP = ml_dtypes.bfloat16
FP8 = mybir.dt.float8e5
FP8_NP = ml_dtypes.float8_e5m2
NPAIR = 4          # K-pairs per step (DoubleRow: 2 tag-blocks per matmul)

SEQ = 16384
TAG = 1024
P = 128            # partitions / PE tile edge / chains per core
NT = TAG // P      # 8 tag tiles
NCORES = 8
L = 16             # chunk length (steps per chunk)
K = 0              # warm-up steps per chain (chunk-start norm is exactly TAG)
LEN = L + K        # lockstep steps per core
DELTA = 8.0        # per-step log-growth folded into Mhat
CHAINS = SEQ // L  # 1024 global chains

# ft DMA chunks: step ranges whose FE tiles arrive in one DMA each
FT_CHUNKS = [(0, 2), (2, 5), (5, LEN)]

_compiled = None
LAST_RESULTS = None


def _build_kernel():
    nc = bacc.Bacc(
        "TRN2",
        target_bir_lowering=False,
        debug=False,
        num_devices=NCORES,
    )

    # mh layout is jt-major: block jt holds Mhat[:, jt*128:(jt+1)*128] as
    # [128 (k partition), 8 kt x 128 (j')] so each group's weights arrive in
    # one DMA.  Block 0 rides in `boot` (one DMA covers everything the first
    # matmul group needs: ucol | ones | mh block 0); the all-ones init state
    # is memset on device (chunk 0's exact 16-step prefix runs on the host).
    BOOT_W = NT + 1 + TAG
    boot = nc.declare_dram_parameter("boot", [P, BOOT_W], FP8, isOutput=False)
    mh = nc.declare_dram_parameter("mh", [P, NT * TAG], FP8, isOutput=False)
    ft = nc.declare_dram_parameter("ft", [P, LEN * TAG], FP8, isOutput=False)

    sums = nc.declare_dram_parameter("sums", [1, 4 * P], F32, isOutput=True)

    with tile.TileContext(nc) as tc, ExitStack() as ctx:
        cpool = ctx.enter_context(tc.tile_pool(name="cpool", bufs=1))
        xpool = ctx.enter_context(tc.tile_pool(name="xpool", bufs=2))
        qpool = ctx.enter_context(
            tc.tile_pool(name="qpool", bufs=1, space="PSUM"))
        rpool = ctx.enter_context(
            tc.tile_pool(name="rpool", bufs=1, space="PSUM"))

        boot_t = cpool.tile([P, BOOT_W], FP8)

        # ---- staged input DMAs on two HWDGE queues (SP + Act)
        nc.sync.dma_start(boot_t[:], boot[:])
        ucol_t = boot_t[:, 0:NT]
        ones_t = boot_t[:, NT:NT + 1]
        # warm-up operand first so the PE dummies start immediately
        warm_sb = cpool.tile([P, P], BF16)
        nc.vector.memset(warm_sb[:], 0.0)

        ft_t = []                     # one tile per chunk
        ft_of = {}                    # step -> (tile, col offset)
        for ci, (s0, s1) in enumerate(FT_CHUNKS):
            tchunk = cpool.tile([P, (s1 - s0) * TAG], FP8, tag=f"ft{ci}",
                                name=f"ft{ci}")
            ft_t.append(tchunk)
            for s in range(s0, s1):
                ft_of[s] = (tchunk, (s - s0) * TAG)

        mh_t = [boot_t[:, NT + 1:NT + 1 + TAG]]
        mh_rest = [cpool.tile([P, TAG], FP8, tag=f"mh{jt}", name=f"mh{jt}")
                   for jt in range(1, NT)]
        mh_t.extend(mh_rest)
        # arrival order: ft chunk 0 IS the initial state (step 0 is folded
        # into it on the host: X_1 = S * fe_0 with S = Mhat column sums), so
        # it loads right after boot; mh blocks alternate across the SP and
        # Act queues so real hardware loads them in parallel.
        nc.scalar.dma_start(ft_t[0][:], ft[:, 0:FT_CHUNKS[0][1] * TAG])
        for jt in range(1, NT):
            eng = nc.sync if jt % 2 == 1 else nc.scalar
            eng.dma_start(mh_t[jt][:], mh[:, jt * TAG:(jt + 1) * TAG])
        for ci in range(1, len(FT_CHUNKS)):
            s0, s1 = FT_CHUNKS[ci]
            nc.scalar.dma_start(ft_t[ci][:], ft[:, s0 * TAG:s1 * TAG])

        # ---- PE pre-warm: dummy matmuls with no DMA deps keep the PE busy
        # through the boot DMA so the pstate ramp completes before step 0.
        warm_ps = rpool.tile([P, P], F32, tag="warm")
        for _ in range(28):
            nc.tensor.matmul(warm_ps[:], lhsT=warm_sb[:], rhs=warm_sb[:],
                             start=True, stop=True)

        rec_slot = {LEN - 1: 2}

        DR = mybir.MatmulPerfMode.DoubleRow
        HB = P // 2    # chains per stream

        def pairs_of(ap2d):
            return ap2d.rearrange("a (two f) -> a two f", two=2)

        # Two interleaved chain-streams (b 0..63 / 64..127): each stream's
        # X'=q*fe mul (ONE fused [128,512] DVE op) hides under the other
        # stream's PE matmuls.  Per-stream state tile layout: [blk(8) x 64].
        # The post-step-0 state is the (host-prescaled) ft chunk 0 itself.
        xt_s = [ft_t[0][:, 0:NT * HB], ft_t[0][:, NT * HB:2 * NT * HB]]

        def pair_view(xs, p):
            return pairs_of(xs[:, 2 * p * HB:2 * (p + 1) * HB])

        for s in range(1, LEN):
            fch, fo = ft_of[s]
            nxt = [None, None]
            for strm in range(2):
                q = qpool.tile([P, NT * HB], F32, tag=f"q{strm}",
                               name=f"q{strm}", bufs=2)
                for jt in range(NT):
                    for p in range(NPAIR):
                        nc.tensor.matmul(
                            q[:, jt * HB:(jt + 1) * HB],
                            lhsT=pairs_of(
                                mh_t[jt][:, 2 * p * P:2 * (p + 1) * P]),
                            rhs=pair_view(xt_s[strm], p),
                            start=(p == 0), stop=(p == NPAIR - 1),
                            perf_mode=DR)
                xq = xpool.tile([P, NT * HB], FP8, tag=f"xq{strm}",
                                name=f"xq{strm}")
                # host ft layout [s][strm][blk(8)][64] -> one contiguous slice
                ftv = fch[:, fo + strm * NT * HB:fo + (strm + 1) * NT * HB]
                nc.vector.tensor_mul(xq[:], q[:], ftv)
                nxt[strm] = xq
            xt_s = nxt
            if s in rec_slot:
                rec = rpool.tile([1, P], F32, tag="rec")
                for strm in range(2):
                    for kt in range(NT):
                        nc.tensor.matmul(
                            rec[:, strm * HB:(strm + 1) * HB],
                            lhsT=ones_t[:],
                            rhs=nxt[strm][:, kt * HB:(kt + 1) * HB],
                            start=(kt == 0), stop=(kt == NT - 1))
                slot = rec_slot[s]
                rec_sb = cpool.tile([1, P], F32, tag="rec_sb", name="rec_sb")
                nc.scalar.copy(rec_sb[:], rec[:])
                nc.sync.dma_start(sums[:, slot * P:(slot + 1) * P], rec_sb[:])

        dots = rpool.tile([1, P], F32, tag="dots")
        for strm in range(2):
            for kt in range(NT):
                nc.tensor.matmul(
                    dots[:, strm * HB:(strm + 1) * HB],
                    lhsT=ucol_t[:, kt:kt + 1],
                    rhs=xt_s[strm][:, kt * HB:(kt + 1) * HB],
                    start=(kt == 0), stop=(kt == NT - 1))
        dots_sb = cpool.tile([1, P], F32)
        nc.scalar.copy(dots_sb[:], dots[:])
        nc.scalar.dma_start(sums[:, 3 * P:4 * P], dots_sb[:])

    nc.compile()
    return nc


def _prep_inputs(feats, T, start_i):
    """Host-side: Mhat (jt-major blocks), pre-exp'd per-step feat tiles,
    init state."""
    mhat = np.exp(T.T.astype(np.float64) - DELTA).astype(np.float32)  # [j, j']
    # block jt: [128 (k part), NT kt x 128] with element [i, kt*128 + c] =
    # Mhat[kt*128 + i, jt*128 + c]
    mh_sb = np.ascontiguousarray(
        mhat.reshape(NT, P, NT, P)      # [kt, i, jt, c]
        .transpose(1, 2, 0, 3)          # [i, jt, kt, c]
        .reshape(P, NT * TAG)).astype(FP8_NP)

    fe = np.exp(feats.astype(np.float32)).astype(FP8_NP)  # [SEQ, TAG]
    # step 0 is folded on the host: X_1[j',b] = S[j'] * fe_0[j',b] with
    # S = Mhat column sums, multiplied in f32 before the fp8 quantization.
    S = mhat.sum(axis=0).astype(np.float32)               # [1024 (j')]

    in_maps = []
    for g in range(NCORES):
        # chain c = 128g + b covers rows [16c, 16c+16)
        b = np.arange(P)
        rows = (L * (P * g + b))[None, :] + np.arange(LEN)[:, None]
        ftg = fe[rows]                                  # [LEN, 128(b), 1024(j)]
        ftg = ftg.transpose(0, 2, 1)                    # [s, j, b]
        ftg[0] = (np.exp(feats[rows[0]].astype(np.float32).T)
                  * S[:, None]).astype(FP8_NP)
        ftg = ftg.reshape(LEN, NT, P, P)                # [s, jt, j_l, b]
        # device layout: [j_l part, s, strm(2), blk(8), b_local(64)]
        HB = P // 2
        ftg = ftg.reshape(LEN, NT, P, 2, HB)      # [s, blk, j_l, strm, bl]
        ft_sb = np.ascontiguousarray(
            ftg.transpose(2, 0, 3, 1, 4).reshape(P, LEN * TAG))

        in_maps.append({"mh": mh_sb, "ft": ft_sb})
    return in_maps


def kernel(feats, transitions, tags, start_idx, stop_idx):
    global _compiled, LAST_RESULTS
    feats = np.ascontiguousarray(np.asarray(feats, dtype=np.float32))
    T = np.ascontiguousarray(np.asarray(transitions, dtype=np.float32))
    tags_np = np.asarray(tags).astype(np.int64)
    start_i = int(np.asarray(start_idx))
    stop_i = int(np.asarray(stop_idx))

    in_maps = _prep_inputs(feats, T, start_i)
    u = np.exp(T[stop_i].astype(np.float64)).astype(np.float32)
    ucol_sb = np.ascontiguousarray(u.reshape(NT, P).T).astype(FP8_NP)
    ones_sb = np.ones((P, 1), FP8_NP)
    for m in in_maps:
        # boot = ucol | ones | mh block 0
        m["boot"] = np.ascontiguousarray(np.concatenate(
            [ucol_sb, ones_sb, m["mh"][:, 0:TAG]], axis=1))

    # chunk 0's exact 16-step prefix in f64 on the host (16 matvecs):
    # anchors the absolute scale that all other chunks telescope from.
    E64 = np.exp(T.astype(np.float64))
    w = np.zeros(TAG, np.float64)
    w[start_i] = 1.0
    fe64 = np.exp(feats[:L].astype(np.float64))
    for t in range(L):
        w = fe64[t] * (E64 @ w)
    logw16 = float(np.log(w.sum()))

    if _compiled is None:
        _compiled = _build_kernel()
    res = run_bass_kernel_spmd(
        _compiled, in_maps, list(range(NCORES)),
        trace=bool(os.environ.get("KERNEL_TRACE")))
    LAST_RESULTS = res
    results = res.results

    # ---- host stitch (~3k scalars)
    sums_by_core = [results[g]["sums"].reshape(4, P) for g in range(NCORES)]
    end = np.concatenate(
        [sums_by_core[g][2] for g in range(NCORES)]).astype(np.float64)
    d = float(sums_by_core[NCORES - 1][3][P - 1])

    # chunk-start norm is exactly |ones| = TAG (zero warm-up steps)
    fs = (np.log(d) - np.log(end[CHAINS - 1])
          + float(np.sum(np.log(end[1:]))) - (CHAINS - 1) * np.log(float(TAG))
          + logw16 + (SEQ - L) * DELTA)

    # ---- gold score on host (index gathers, O(seq + tag))
    tags_ext = np.concatenate([np.array([start_i], dtype=np.int64), tags_np])
    gold = (float(T[tags_ext[1:], tags_ext[:-1]].astype(np.float64).sum())
            + feats[tags_ext[1:]].astype(np.float64).sum(axis=0)
            + float(T[stop_i, tags_ext[-1]]))

    return (fs - gold).astype(np.float32)
